# revision 35
# baseline (speedup 1.0000x reference)
"""Trainium2 Bass kernel for DescriptorMatchLoss (retrieval_knn).

Reference:
    d2[i,j,n,m] = ||denorm(pts_src[i,n]) - denorm(pts_dst[i,j,m])||^2
    mask        = d2 <= 8^2
    cos[i,j,n,m] = <fhat[j,n], fhat[i,m]>
    loss = sum(mask * (1 - cos)) / max(sum(mask), 1)

Strategy (v2, window-pruned):
  * The mask is geometrically sparse (matches need pixel distance <= 8 in a
    640x480 image; ~6.5e-4 density).  Host sorts src points (n axis) and dst
    points (m axis) of every pair by x; each 128-point n-slab then only
    matches a narrow contiguous m-window (~10% of the full [N,N] grid).
  * Only those windows are computed on device:
      z = 64 - d2  via a K=14 bf16 geometry matmul (hi/lo split => exact),
      dots = <fj, fi> via a K=64 fp8 matmul of JL-projected unit features
      (random orthonormal projection 256->64; adds ~5e-4 rel err, gate 2e-2),
      count: ACT Sign(z) with fused accumulation (+-1 convention, corrected
      exactly on host using the static window sizes),
      masked sum: one DVE scalar_tensor_tensor (z >= 0) * dots with fused
      accumulation ({0,1} convention => exact, no correction).
  * Per-pair window offsets live in host-gathered input tensors
    (fiT_win/geoR_win are [slot, nt, :, Wmax] gathers), so the compiled
    graph is identical across the 8 cores (SPMD) while every core works on
    its own tight windows.  Padding columns are real points provably
    outside radius, so they self-mask.
  * 8 cores x 2 pairs; host reduces the per-segment count/sum columns.

kernel(**inputs) takes FULL inputs, returns the scalar loss (fp32).
"""

import sys

for _p in ("/opt/pypackages", "/opt/trn_rl_repo"):
    if _p not in sys.path:
        sys.path.insert(0, _p)

import numpy as np
import ml_dtypes

BF16 = ml_dtypes.bfloat16
FP8 = ml_dtypes.float8_e4m3

# Problem constants (hardcoded per contract).
B, N, D = 4, 2048, 256
HEIGHT, WIDTH = 480, 640
RADIUS = 8.0
RADIUS2 = 64.0
N_CORES = 8
Q = (B * B) // N_CORES  # pair slots per core (2)

P = 128          # partitions
NT = N // P      # 16 n-slabs of 128
KGEO = 14        # geometry contraction rows
KD = 64          # JL-projected feature dim
SEGW = 512       # max segment width (one PSUM bank of f32)
WIN_EPS = 0.05   # window dilation in px (covers bf16-split rounding)

# Tunables.
Z_BUFS = 4       # PSUM buffers cycling for z tiles
DOT_BUFS = 4     # PSUM buffers cycling for dots tiles
WIDE = 1         # bins per segment (1 = 512-wide tiles, 2 = 1024-wide)
SCR_BUFS = 3
STT_DUAL_PSUM = False  # DVE STT reads z (PSUM) and dots (PSUM) directly;
                       # False: ACT Sign -> SBUF, DVE STT(sgn, dots), host
                       # corrects the ext sums (+-1 convention)
DOTS_QUAD = False      # replicate fjT/fiT at 2 PE row groups (conc. dots)
DOTS_BASE = 64         # SBUF base partition of fjT/fiT: dots run in PE rows
                       # [64,128), disjoint from the d2 row groups
DOTS_DR = False        # fp8 DoubleRow dots: K=64 packed as 32 partitions x 2
                       # interleaved rows; halves the PE moving-stream cols
D2_QUAD = True         # replicate geo at PE row groups (concurrent d2)
D2_NGRP = 2            # d2 row groups (rows [0,32) and [32,64))

_CACHE = {}
LAST = None  # BassKernelResults of the most recent run (for test harness)


# ---------------------------------------------------------------------------
# Host-side math

def _split2(x):
    hi = x.astype(BF16)
    lo = (x - hi.astype(np.float64)).astype(BF16)
    return hi, lo


def _split3(x):
    hi = x.astype(BF16)
    r = x - hi.astype(np.float64)
    mid = r.astype(BF16)
    lo = (r - mid.astype(np.float64)).astype(BF16)
    return hi, mid, lo


def _jl_matrix():
    if "jl" not in _CACHE:
        rng = np.random.default_rng(12345)
        G = rng.standard_normal((D, KD))
        Qm, _ = np.linalg.qr(G)
        _CACHE["jl"] = Qm * np.sqrt(D / KD)
    return _CACHE["jl"]


def _schedule(widths):
    """Greedy bin-pack per-nt windows into segments of total width <= SEGW.

    widths: [NT] ints.  Returns list of segments; each segment is a list of
    blocks (nt, wlo, wlen, col_off) where wlo is the offset inside the
    (padded) window of that nt.
    """
    segs = []
    cur = []
    curw = 0
    for nt in range(NT):
        w = int(widths[nt])
        wlo = 0
        while w > 0:
            if curw == SEGW:
                segs.append(cur)
                cur, curw = [], 0
            take = min(w, SEGW - curw)
            cur.append((nt, wlo, take, curw))
            curw += take
            wlo += take
            w -= take
    if cur:
        segs.append(cur)
    return segs


def _host_prep(features, pts_src, pts_dst, height, width):
    """Build per-core device inputs + the static schedule."""
    height = int(height)
    width = int(width)
    scale32 = np.array(
        [(width - 1) * 0.5, (height - 1) * 0.5], dtype=np.float32
    )

    # Match the reference's fp32 denormalization rounding, then center.
    ps32 = (pts_src.astype(np.float32) + np.float32(1.0)) * scale32  # [B,N,2]
    pd32 = (pts_dst.astype(np.float32) + np.float32(1.0)) * scale32  # [B,B,N,2]
    psc = ps32.astype(np.float64) - scale32.astype(np.float64)
    pdc = pd32.astype(np.float64) - scale32.astype(np.float64)

    # Sort n by x per src batch i; m by x per (i, j).
    pi = [np.argsort(psc[i, :, 0], kind="stable") for i in range(B)]
    sg = [[np.argsort(pdc[i, j, :, 0], kind="stable") for j in range(B)]
          for i in range(B)]
    psc_s = np.stack([psc[i][pi[i]] for i in range(B)])          # [B,N,2]
    pdc_s = np.stack(
        [np.stack([pdc[i, j][sg[i][j]] for j in range(B)]) for i in range(B)]
    )                                                            # [B,B,N,2]

    # Geometry split (z = 64 - d2 = 2 p.q + (64 - s_src) - s_dst).
    phx, plx = _split2(psc_s[..., 0])
    phy, ply = _split2(psc_s[..., 1])
    qhx, qlx = _split2(pdc_s[..., 0])
    qhy, qly = _split2(pdc_s[..., 1])
    sh, sm, sl = _split3(
        RADIUS2
        - (
            (phx.astype(np.float64) + plx.astype(np.float64)) ** 2
            + (phy.astype(np.float64) + ply.astype(np.float64)) ** 2
        )
    )  # [B,N]
    tq = (
        (qhx.astype(np.float64) + qlx.astype(np.float64)) ** 2
        + (qhy.astype(np.float64) + qly.astype(np.float64)) ** 2
    )
    th, tm, tl = _split3(tq)  # [B,B,N]

    ones_bn = np.ones((B, N), dtype=BF16)
    ones_bbn = np.ones((B, B, N), dtype=BF16)
    neg_ones_bn = -ones_bn
    p2hx = (2.0 * phx.astype(np.float64)).astype(BF16)
    p2lx = (2.0 * plx.astype(np.float64)).astype(BF16)
    p2hy = (2.0 * phy.astype(np.float64)).astype(BF16)
    p2ly = (2.0 * ply.astype(np.float64)).astype(BF16)
    geoL_all = np.stack(
        [p2hx, p2hx, p2lx, p2lx, p2hy, p2hy, p2ly, p2ly,
         sh, sm, sl, neg_ones_bn, neg_ones_bn, neg_ones_bn],
        axis=1,
    )  # [B, 14, N]  (n sorted by pi[i])
    geoR_all = np.stack(
        [qhx, qlx, qhx, qlx, qhy, qly, qhy, qly,
         ones_bbn, ones_bbn, ones_bbn, th, tm, tl],
        axis=2,
    )  # [B, B, 14, N]  (m sorted by sg[i][j])

    # JL-projected, fp8-quantized unit features.
    f64 = features.astype(np.float64)
    fhat = f64 / np.sqrt((f64 * f64).sum(-1, keepdims=True))
    fproj = (fhat @ _jl_matrix()).astype(FP8)   # [B, N, KD]

    # Per-(pair, nt) m-windows in each pair's own sorted index space.
    pair_list = [(p // B, p % B) for p in range(B * B)]
    lo_idx = np.zeros((B * B, NT), dtype=np.int64)
    wid = np.zeros((B * B, NT), dtype=np.int64)
    for p, (i_, j_) in enumerate(pair_list):
        xs = psc_s[i_, :, 0]
        xd = pdc_s[i_, j_, :, 0]
        for nt in range(NT):
            lo = np.searchsorted(xd, xs[nt * P] - RADIUS - WIN_EPS, "left")
            hi = np.searchsorted(
                xd, xs[(nt + 1) * P - 1] + RADIUS + WIN_EPS, "right"
            )
            lo_idx[p, nt] = lo
            wid[p, nt] = hi - lo

    # Uniform (max over pairs) window widths -> one graph for all cores.
    widths = wid.max(axis=0)                     # [NT]
    widths = np.maximum(widths, 1)
    wmax = int(widths.max())
    # Clamp per-pair offsets so windows stay in range; padding columns are
    # real points strictly beyond radius, so they contribute zero mask.
    offs = np.minimum(lo_idx, N - widths[None, :])  # [B*B, NT]

    bins = _schedule(widths)
    if WIDE > 1:
        # Fuse WIDE consecutive bins into one segment; bin k's blocks sit at
        # column offset SEGW*k (blocks never straddle a 512-col PSUM bank).
        segs = []
        for b0 in range(0, len(bins), WIDE):
            blocks = []
            for k, b in enumerate(bins[b0 : b0 + WIDE]):
                blocks += [
                    (nt, wlo, wlen, SEGW * k + coff)
                    for (nt, wlo, wlen, coff) in b
                ]
            segs.append(blocks)
    else:
        segs = bins
    seg_meta = []  # per device segment: (slot q, [(nt, wlo, wlen, coff)], W)
    for q in range(Q):
        for s in segs:
            seg_meta.append((q, s, max(b[3] + b[2] for b in s)))
    nseg = len(seg_meta)

    # Gather per-core inputs.
    in_maps = []
    for c in range(N_CORES):
        pairs = [Q * c + k for k in range(Q)]
        fjT = np.zeros((Q, KD, N), dtype=FP8)
        fiT_win = np.zeros((Q, NT, KD, wmax), dtype=FP8)
        geoR_win = np.zeros((Q, NT, KGEO, wmax), dtype=BF16)
        geoL = np.zeros((Q, KGEO, N), dtype=BF16)
        for k, pnum in enumerate(pairs):
            i_, j_ = pair_list[pnum]
            fjT[k] = fproj[j_][pi[i_]].T          # n sorted by pi[i]
            geoL[k] = geoL_all[i_]
            fi_s = fproj[i_][sg[i_][j_]]          # [N, KD] m-sorted
            gR = geoR_all[i_, j_]                 # [14, N]
            for nt in range(NT):
                o = int(offs[pnum, nt])
                w = int(widths[nt])
                fiT_win[k, nt, :, :w] = fi_s[o : o + w].T
                geoR_win[k, nt, :, :w] = gR[:, o : o + w]
        in_maps.append(
            {
                "fjT": fjT,
                "fiT": np.ascontiguousarray(fiT_win),
                "geoR": np.ascontiguousarray(geoR_win),
                "geoL": geoL,
            }
        )
    sched = {
        "widths": tuple(int(w) for w in widths),
        "wmax": wmax,
        "segs": tuple(
            (q, tuple(blocks), w) for (q, blocks, w) in seg_meta
        ),
        "nseg": nseg,
    }
    return in_maps, sched


# ---------------------------------------------------------------------------
# Device kernel

def _build_bass(sched, reps=1, mode="full", split_waits=True):
    import concourse.bass as bass
    import concourse.mybir as mybir
    import concourse.tile as tile

    nc = bass.Bass(trn_type="TRN2", target_bir_lowering=False, debug=False)
    f32 = mybir.dt.float32
    bf16 = mybir.dt.bfloat16
    fp8 = mybir.dt.float8e4

    wmax = sched["wmax"]
    segs = sched["segs"]
    nseg = sched["nseg"]

    fjT_d = nc.dram_tensor("fjT", [Q, KD, N], fp8, kind="ExternalInput")
    fiT_d = nc.dram_tensor(
        "fiT", [Q, NT, KD, wmax], fp8, kind="ExternalInput"
    )
    geoR_d = nc.dram_tensor(
        "geoR", [Q, NT, KGEO, wmax], bf16, kind="ExternalInput"
    )
    geoL_d = nc.dram_tensor("geoL", [Q, KGEO, N], bf16, kind="ExternalInput")
    out_d = nc.dram_tensor("out", [P, 2 * nseg], f32, kind="ExternalOutput")

    with tile.TileContext(nc) as tc:
        with (
            tc.tile_pool(name="feat", bufs=1) as feat_pool,
            tc.tile_pool(name="geo", bufs=1) as geo_pool,
            tc.tile_pool(name="acc", bufs=1) as acc_pool,
            tc.tile_pool(name="scr", bufs=SCR_BUFS) as scr_pool,
            tc.tile_pool(name="psum_z", bufs=Z_BUFS, space="PSUM") as z_pool,
            tc.tile_pool(name="psum_d", bufs=DOT_BUFS, space="PSUM") as d_pool,
        ):
            dgrp = 2 if DOTS_QUAD else 1
            dbase = DOTS_BASE
            if DOTS_DR:
                # d = r*32 + p packing on both operands (any consistent
                # bijection works; the PE pairs stationary row (p, r) with
                # moving (p, r)).
                fjT_sb = feat_pool.tile([dbase + KD // 2, Q, 2, N], fp8)
                fiT_sb = feat_pool.tile(
                    [dbase + KD // 2, Q, NT, 2, wmax], fp8
                )
            else:
                fjT_sb = feat_pool.tile(
                    [dbase + KD if dbase else KD, Q, N], fp8
                )
                fiT_sb = feat_pool.tile(
                    [dbase + KD if dbase else KD, Q, NT, wmax], fp8
                )
            ngrp = D2_NGRP if D2_QUAD else 1
            geoL_sb = geo_pool.tile([P if D2_QUAD else KGEO, Q, N], bf16)
            geoR_sb = geo_pool.tile(
                [P if D2_QUAD else KGEO, Q, NT, wmax], bf16
            )
            cnt_acc = acc_pool.tile([P, nseg], f32)
            ext_acc = acc_pool.tile([P, nseg], f32)
            if mode != "full":
                nc.vector.memset(cnt_acc[:], 0.0)
                nc.vector.memset(ext_acc[:], 0.0)

            # Input DMAs (slot-major so slot-0 compute can start early).
            for rg in range(ngrp):
                nc.sync.dma_start(
                    out=geoL_sb[32 * rg : 32 * rg + KGEO, :, :],
                    in_=geoL_d[:].rearrange("q k n -> k q n"),
                )
                nc.sync.dma_start(
                    out=geoR_sb[32 * rg : 32 * rg + KGEO, :, :, :],
                    in_=geoR_d[:].rearrange("q t k w -> k q t w"),
                )
            if DOTS_DR:
                nc.sync.dma_start(
                    out=fjT_sb[dbase : dbase + KD // 2, :, :, :],
                    in_=fjT_d[:].rearrange("q (r p) n -> p q r n", p=KD // 2),
                )
                nc.sync.dma_start(
                    out=fiT_sb[dbase : dbase + KD // 2, :, :, :, :],
                    in_=fiT_d[:].rearrange(
                        "q t (r p) w -> p q t r w", p=KD // 2
                    ),
                )
            else:
                for dg in range(dgrp):
                    o = dbase + KD * dg
                    nc.sync.dma_start(
                        out=fjT_sb[o : o + KD, :, :],
                        in_=fjT_d[:].rearrange("q k n -> k q n"),
                    )
                    nc.sync.dma_start(
                        out=fiT_sb[o : o + KD, :, :, :],
                        in_=fiT_d[:].rearrange("q t k w -> k q t w"),
                    )

            # No DMA-tick absorbers: the first z matmul should start as soon
            # as the (small) geometry DMAs land, without waiting for the
            # larger fjT/fiT transfers; the first dots matmul carries those
            # waits naturally (hoisted by _split_multi_waits where needed).
            if mode == "noop":
                dummy_sb = scr_pool.tile([1, 8], f32, tag="dmy")
                nc.vector.memset(dummy_sb[:], 0.0)

            def emit_seg(si, q, blocks, W):
                z_ps = z_pool.tile([P, W], f32, tag="z")
                d_ps = d_pool.tile([P, W], f32, tag="d")
                if mode in ("full", "d2only", "zvec"):
                    # One PE row group per segment: matmuls within a segment
                    # share a PSUM bank (must be serial); consecutive
                    # segments use different row groups and banks, so their
                    # d2 streams overlap on the PE array.
                    rg = (si % ngrp) * 32
                    for nt, wlo, wlen, coff in blocks:
                        nc.tensor.matmul(
                            z_ps[:, coff : coff + wlen],
                            geoL_sb[rg : rg + KGEO, q, nt * P : (nt + 1) * P],
                            geoR_sb[rg : rg + KGEO, q, nt, wlo : wlo + wlen],
                            start=True,
                            stop=True,
                            tile_position=(rg, 0),
                        )
                if mode in ("full", "dotsonly"):
                    if DOTS_DR:
                        for nt, wlo, wlen, coff in blocks:
                            nc.tensor.matmul(
                                d_ps[:, coff : coff + wlen],
                                fjT_sb[dbase : dbase + KD // 2, q, 0:2,
                                       nt * P : (nt + 1) * P],
                                fiT_sb[dbase : dbase + KD // 2, q, nt, 0:2,
                                       wlo : wlo + wlen],
                                start=True,
                                stop=True,
                                perf_mode=mybir.MatmulPerfMode.DoubleRow,
                            )
                    else:
                        dg = dbase + (si % dgrp) * KD
                        for nt, wlo, wlen, coff in blocks:
                            nc.tensor.matmul(
                                d_ps[:, coff : coff + wlen],
                                fjT_sb[dg : dg + KD, q,
                                       nt * P : (nt + 1) * P],
                                fiT_sb[dg : dg + KD, q, nt,
                                       wlo : wlo + wlen],
                                start=True,
                                stop=True,
                            )
                if mode in ("full", "zvec"):
                    sgn_scr = scr_pool.tile([P, W], bf16, tag="sgn")
                    nc.scalar.activation(
                        sgn_scr[:],
                        z_ps[:],
                        mybir.ActivationFunctionType.Sign,
                        accum_out=cnt_acc[:, si : si + 1],
                    )
                if mode == "full":
                    stt_scr = scr_pool.tile([P, W], bf16, tag="stt")
                    if STT_DUAL_PSUM:
                        nc.vector.scalar_tensor_tensor(
                            out=stt_scr[:],
                            in0=z_ps[:],
                            scalar=0.0,
                            in1=d_ps[:],
                            op0=mybir.AluOpType.is_ge,
                            op1=mybir.AluOpType.mult,
                            accum_out=ext_acc[:, si : si + 1],
                        )
                    else:
                        nc.vector.scalar_tensor_tensor(
                            out=stt_scr[:],
                            in0=sgn_scr[:],
                            scalar=0.0,
                            in1=d_ps[:],
                            op0=mybir.AluOpType.bypass,
                            op1=mybir.AluOpType.mult,
                            accum_out=ext_acc[:, si : si + 1],
                        )

            def emit_body():
                if mode == "noop":
                    nc.scalar.copy(dummy_sb[0:1, 2:3], dummy_sb[0:1, 0:1])
                    return
                for si, (q, blocks, W) in enumerate(segs):
                    emit_seg(si, q, blocks, W)

            if reps == 1:
                emit_body()
            else:
                with tc.For_i(0, reps, 1):
                    emit_body()

            nc.sync.dma_start(out=out_d[:, 0:nseg], in_=cnt_acc[:])
            nc.sync.dma_start(out=out_d[:, nseg : 2 * nseg], in_=ext_acc[:])

    if split_waits:
        _split_multi_waits(nc)
    return nc


def _split_multi_waits(nc):
    """Walrus rejects >1 sync-wait on compute/DMA instruction encodings.

    Hoist all but one wait of any multi-wait instruction onto standalone
    InstEventSemaphore instructions inserted immediately before it on the
    same engine queue.
    """
    import concourse.mybir as mybir

    n_split = 0
    for bb in nc.main_func.blocks:
        new_list = []
        for inst in bb.instructions:
            si = inst.sync_info
            if (
                si is not None
                and si.on_wait
                and len(si.on_wait) > 1
                and not isinstance(inst, mybir.InstEventSemaphore)
            ):
                waits = list(si.on_wait)
                for k, w in enumerate(waits[:-1]):
                    n_split += 1
                    new_list.append(
                        mybir.InstEventSemaphore(
                            name=f"{inst.name}-hw{k}",
                            engine=inst.engine,
                            ins=[],
                            outs=[],
                            sync_info=mybir.SyncInfo(on_wait=[w], on_update=[]),
                        )
                    )
                inst.sync_info = mybir.SyncInfo(
                    on_wait=[waits[-1]], on_update=list(si.on_update or [])
                )
            new_list.append(inst)
        bb.instructions[:] = new_list
    return n_split


def _get_bass(sched):
    key = ("nc", sched["segs"], STT_DUAL_PSUM)
    if key not in _CACHE:
        _CACHE[key] = _build_bass(sched)
    return _CACHE[key]


def _combine(results, sched, in_maps=None):
    segs = sched["segs"]
    nseg = sched["nseg"]
    a_tot = 0.0
    b_tot = 0.0
    for c, res in enumerate(results):
        out = res["out"].astype(np.float64)
        cnt = out[:, 0:nseg].sum(axis=0)
        ext = out[:, nseg : 2 * nseg].sum(axis=0)
        for si, (q, blocks, W) in enumerate(segs):
            a_tot += 0.5 * (cnt[si] + P * W)
            if STT_DUAL_PSUM:
                b_tot += ext[si]
            else:
                # +-1 ext convention: b = 0.5*(ext + sum_window dots)
                corr = 0.0
                fj = in_maps[c]["fjT"][q].astype(np.float64)   # [KD, N]
                fi = in_maps[c]["fiT"][q].astype(np.float64)   # [NT,KD,wmax]
                for nt, wlo, wlen, coff in blocks:
                    cj = fj[:, nt * P : (nt + 1) * P].sum(axis=1)
                    ci = fi[nt, :, wlo : wlo + wlen].sum(axis=1)
                    corr += float(cj @ ci)
                b_tot += 0.5 * (ext[si] + corr)
    return a_tot, b_tot


def kernel(features, pts_src, pts_dst, invis_idx, height, width):
    global LAST
    del invis_idx  # unused by the reference computation

    features = np.asarray(features)
    pts_src = np.asarray(pts_src)
    pts_dst = np.asarray(pts_dst)

    in_maps, sched = _host_prep(features, pts_src, pts_dst, height, width)

    from concourse.bass_utils import run_bass_kernel_spmd

    nc = _get_bass(sched)
    LAST = run_bass_kernel_spmd(nc, in_maps, core_ids=list(range(N_CORES)))

    a_tot, b_tot = _combine(LAST.results, sched, in_maps)
    loss = (a_tot - b_tot) / max(a_tot, 1.0)
    return np.float32(loss)


# revision 37
# speedup vs baseline: 1.1480x; 1.1480x over previous
"""Trainium2 Bass kernel for DescriptorMatchLoss (retrieval_knn).

Reference:
    d2[i,j,n,m] = ||denorm(pts_src[i,n]) - denorm(pts_dst[i,j,m])||^2
    mask        = d2 <= 8^2
    cos[i,j,n,m] = <fhat[j,n], fhat[i,m]>
    loss = sum(mask * (1 - cos)) / max(sum(mask), 1)

Strategy (v2, window-pruned):
  * The mask is geometrically sparse (matches need pixel distance <= 8 in a
    640x480 image; ~6.5e-4 density).  Host sorts src points (n axis) and dst
    points (m axis) of every pair by x; each 128-point n-slab then only
    matches a narrow contiguous m-window (~10% of the full [N,N] grid).
  * Only those windows are computed on device:
      z = 64 - d2  via a K=14 bf16 geometry matmul (hi/lo split => exact),
      dots = <fj, fi> via a K=64 fp8 matmul of JL-projected unit features
      (random orthonormal projection 256->64; adds ~5e-4 rel err, gate 2e-2),
      count: ACT Sign(z) with fused accumulation (+-1 convention, corrected
      exactly on host using the static window sizes),
      masked sum: one DVE scalar_tensor_tensor (z >= 0) * dots with fused
      accumulation ({0,1} convention => exact, no correction).
  * Per-pair window offsets live in host-gathered input tensors
    (fiT_win/geoR_win are [slot, nt, :, Wmax] gathers), so the compiled
    graph is identical across the 8 cores (SPMD) while every core works on
    its own tight windows.  Padding columns are real points provably
    outside radius, so they self-mask.
  * 8 cores x 2 pairs; host reduces the per-segment count/sum columns.

kernel(**inputs) takes FULL inputs, returns the scalar loss (fp32).
"""

import sys

for _p in ("/opt/pypackages", "/opt/trn_rl_repo"):
    if _p not in sys.path:
        sys.path.insert(0, _p)

import numpy as np
import ml_dtypes

BF16 = ml_dtypes.bfloat16
FP8 = ml_dtypes.float8_e4m3

# Problem constants (hardcoded per contract).
B, N, D = 4, 2048, 256
HEIGHT, WIDTH = 480, 640
RADIUS = 8.0
RADIUS2 = 64.0
N_CORES = 8
Q = (B * B) // N_CORES  # pair slots per core (2)

P = 128          # partitions
NT = N // P      # 16 n-slabs of 128
KGEO = 14        # geometry contraction rows
KD = 64          # JL-projected feature dim
SEGW = 512       # max segment width (one PSUM bank of f32)
WIN_EPS = 0.05   # window dilation in px (covers bf16-split rounding)

# Tunables.
Z_BUFS = 4       # PSUM buffers cycling for z tiles
DOT_BUFS = 4     # PSUM buffers cycling for dots tiles
WIDE = 1         # bins per segment (1 = 512-wide tiles, 2 = 1024-wide)
SCR_BUFS = 3
STT_DUAL_PSUM = False  # DVE STT reads z (PSUM) and dots (PSUM) directly;
                       # False: ACT Sign -> SBUF, DVE STT(sgn, dots), host
                       # corrects the ext sums (+-1 convention)
DOTS_QUAD = False      # replicate fjT/fiT at 2 PE row groups (conc. dots)
DOTS_BASE = 64         # SBUF base partition of fjT/fiT: dots run in PE rows
                       # [64,128), disjoint from the d2 row groups
DOTS_DR = False        # fp8 DoubleRow dots: K=64 packed as 32 partitions x 2
                       # interleaved rows; halves the PE moving-stream cols
D2_QUAD = True         # replicate geo at PE row groups (concurrent d2)
D2_NGRP = 2            # d2 row groups (rows [0,32) and [32,64))

_CACHE = {}
LAST = None  # BassKernelResults of the most recent run (for test harness)


# ---------------------------------------------------------------------------
# Host-side math

def _split2(x):
    hi = x.astype(BF16)
    lo = (x - hi.astype(np.float64)).astype(BF16)
    return hi, lo


def _split3(x):
    hi = x.astype(BF16)
    r = x - hi.astype(np.float64)
    mid = r.astype(BF16)
    lo = (r - mid.astype(np.float64)).astype(BF16)
    return hi, mid, lo


def _jl_matrix():
    if "jl" not in _CACHE:
        rng = np.random.default_rng(12345)
        G = rng.standard_normal((D, KD))
        Qm, _ = np.linalg.qr(G)
        _CACHE["jl"] = Qm * np.sqrt(D / KD)
    return _CACHE["jl"]


def _schedule(widths):
    """Greedy bin-pack per-nt windows into segments of total width <= SEGW.

    widths: [NT] ints.  Returns list of segments; each segment is a list of
    blocks (nt, wlo, wlen, col_off) where wlo is the offset inside the
    (padded) window of that nt.
    """
    segs = []
    cur = []
    curw = 0
    for nt in range(NT):
        w = int(widths[nt])
        wlo = 0
        while w > 0:
            if curw == SEGW:
                segs.append(cur)
                cur, curw = [], 0
            take = min(w, SEGW - curw)
            cur.append((nt, wlo, take, curw))
            curw += take
            wlo += take
            w -= take
    if cur:
        segs.append(cur)
    return segs


def _host_prep(features, pts_src, pts_dst, height, width):
    """Build per-core device inputs + the static schedule."""
    height = int(height)
    width = int(width)
    scale32 = np.array(
        [(width - 1) * 0.5, (height - 1) * 0.5], dtype=np.float32
    )

    # Match the reference's fp32 denormalization rounding, then center.
    ps32 = (pts_src.astype(np.float32) + np.float32(1.0)) * scale32  # [B,N,2]
    pd32 = (pts_dst.astype(np.float32) + np.float32(1.0)) * scale32  # [B,B,N,2]
    psc = ps32.astype(np.float64) - scale32.astype(np.float64)
    pdc = pd32.astype(np.float64) - scale32.astype(np.float64)

    # Sort n by x per src batch i; m by x per (i, j).
    pi = [np.argsort(psc[i, :, 0], kind="stable") for i in range(B)]
    sg = [[np.argsort(pdc[i, j, :, 0], kind="stable") for j in range(B)]
          for i in range(B)]
    psc_s = np.stack([psc[i][pi[i]] for i in range(B)])          # [B,N,2]
    pdc_s = np.stack(
        [np.stack([pdc[i, j][sg[i][j]] for j in range(B)]) for i in range(B)]
    )                                                            # [B,B,N,2]

    # Geometry split (z = 64 - d2 = 2 p.q + (64 - s_src) - s_dst).
    phx, plx = _split2(psc_s[..., 0])
    phy, ply = _split2(psc_s[..., 1])
    qhx, qlx = _split2(pdc_s[..., 0])
    qhy, qly = _split2(pdc_s[..., 1])
    sh, sm, sl = _split3(
        RADIUS2
        - (
            (phx.astype(np.float64) + plx.astype(np.float64)) ** 2
            + (phy.astype(np.float64) + ply.astype(np.float64)) ** 2
        )
    )  # [B,N]
    tq = (
        (qhx.astype(np.float64) + qlx.astype(np.float64)) ** 2
        + (qhy.astype(np.float64) + qly.astype(np.float64)) ** 2
    )
    th, tm, tl = _split3(tq)  # [B,B,N]

    ones_bn = np.ones((B, N), dtype=BF16)
    ones_bbn = np.ones((B, B, N), dtype=BF16)
    neg_ones_bn = -ones_bn
    p2hx = (2.0 * phx.astype(np.float64)).astype(BF16)
    p2lx = (2.0 * plx.astype(np.float64)).astype(BF16)
    p2hy = (2.0 * phy.astype(np.float64)).astype(BF16)
    p2ly = (2.0 * ply.astype(np.float64)).astype(BF16)
    geoL_all = np.stack(
        [p2hx, p2hx, p2lx, p2lx, p2hy, p2hy, p2ly, p2ly,
         sh, sm, sl, neg_ones_bn, neg_ones_bn, neg_ones_bn],
        axis=1,
    )  # [B, 14, N]  (n sorted by pi[i])
    geoR_all = np.stack(
        [qhx, qlx, qhx, qlx, qhy, qly, qhy, qly,
         ones_bbn, ones_bbn, ones_bbn, th, tm, tl],
        axis=2,
    )  # [B, B, 14, N]  (m sorted by sg[i][j])

    # JL-projected, fp8-quantized unit features.
    f64 = features.astype(np.float64)
    fhat = f64 / np.sqrt((f64 * f64).sum(-1, keepdims=True))
    fproj = (fhat @ _jl_matrix()).astype(FP8)   # [B, N, KD]

    # Per-(pair, nt) m-windows in each pair's own sorted index space.
    pair_list = [(p // B, p % B) for p in range(B * B)]
    lo_idx = np.zeros((B * B, NT), dtype=np.int64)
    wid = np.zeros((B * B, NT), dtype=np.int64)
    for p, (i_, j_) in enumerate(pair_list):
        xs = psc_s[i_, :, 0]
        xd = pdc_s[i_, j_, :, 0]
        for nt in range(NT):
            lo = np.searchsorted(xd, xs[nt * P] - RADIUS - WIN_EPS, "left")
            hi = np.searchsorted(
                xd, xs[(nt + 1) * P - 1] + RADIUS + WIN_EPS, "right"
            )
            lo_idx[p, nt] = lo
            wid[p, nt] = hi - lo

    # Uniform (max over pairs) window widths -> one graph for all cores.
    widths = wid.max(axis=0)                     # [NT]
    widths = np.maximum(widths, 1)
    wmax = int(widths.max())
    # Clamp per-pair offsets so windows stay in range; padding columns are
    # real points strictly beyond radius, so they contribute zero mask.
    offs = np.minimum(lo_idx, N - widths[None, :])  # [B*B, NT]

    bins = _schedule(widths)
    if WIDE > 1:
        # Fuse WIDE consecutive bins into one segment; bin k's blocks sit at
        # column offset SEGW*k (blocks never straddle a 512-col PSUM bank).
        segs = []
        for b0 in range(0, len(bins), WIDE):
            blocks = []
            for k, b in enumerate(bins[b0 : b0 + WIDE]):
                blocks += [
                    (nt, wlo, wlen, SEGW * k + coff)
                    for (nt, wlo, wlen, coff) in b
                ]
            segs.append(blocks)
    else:
        segs = bins
    seg_meta = []  # per device segment: (slot q, [(nt, wlo, wlen, coff)], W)
    for q in range(Q):
        for s in segs:
            seg_meta.append((q, s, max(b[3] + b[2] for b in s)))
    nseg = len(seg_meta)

    # Gather per-core inputs.
    in_maps = []
    for c in range(N_CORES):
        pairs = [Q * c + k for k in range(Q)]
        fjT = np.zeros((Q, KD, N), dtype=FP8)
        fiT_win = np.zeros((Q, NT, KD, wmax), dtype=FP8)
        geoR_win = np.zeros((Q, NT, KGEO, wmax), dtype=BF16)
        geoL = np.zeros((Q, KGEO, N), dtype=BF16)
        for k, pnum in enumerate(pairs):
            i_, j_ = pair_list[pnum]
            fjT[k] = fproj[j_][pi[i_]].T          # n sorted by pi[i]
            geoL[k] = geoL_all[i_]
            fi_s = fproj[i_][sg[i_][j_]]          # [N, KD] m-sorted
            gR = geoR_all[i_, j_]                 # [14, N]
            for nt in range(NT):
                o = int(offs[pnum, nt])
                w = int(widths[nt])
                fiT_win[k, nt, :, :w] = fi_s[o : o + w].T
                geoR_win[k, nt, :, :w] = gR[:, o : o + w]
        in_maps.append(
            {
                "fjT": fjT,
                "fiT": np.ascontiguousarray(fiT_win),
                "geoR": np.ascontiguousarray(geoR_win),
                "geoL": geoL,
            }
        )
    sched = {
        "widths": tuple(int(w) for w in widths),
        "wmax": wmax,
        "segs": tuple(
            (q, tuple(blocks), w) for (q, blocks, w) in seg_meta
        ),
        "nseg": nseg,
    }
    return in_maps, sched


# ---------------------------------------------------------------------------
# Device kernel

def _build_bass(sched, reps=1, mode="full", split_waits=True):
    import concourse.bass as bass
    import concourse.mybir as mybir
    import concourse.tile as tile

    nc = bass.Bass(trn_type="TRN2", target_bir_lowering=False, debug=False)
    f32 = mybir.dt.float32
    bf16 = mybir.dt.bfloat16
    fp8 = mybir.dt.float8e4

    wmax = sched["wmax"]
    segs = sched["segs"]
    nseg = sched["nseg"]

    fjT_d = nc.dram_tensor("fjT", [Q, KD, N], fp8, kind="ExternalInput")
    fiT_d = nc.dram_tensor(
        "fiT", [Q, NT, KD, wmax], fp8, kind="ExternalInput"
    )
    geoR_d = nc.dram_tensor(
        "geoR", [Q, NT, KGEO, wmax], bf16, kind="ExternalInput"
    )
    geoL_d = nc.dram_tensor("geoL", [Q, KGEO, N], bf16, kind="ExternalInput")
    out_d = nc.dram_tensor("out", [P, 2 * nseg], f32, kind="ExternalOutput")

    with tile.TileContext(nc) as tc:
        with (
            tc.tile_pool(name="feat", bufs=1) as feat_pool,
            tc.tile_pool(name="geo", bufs=1) as geo_pool,
            tc.tile_pool(name="acc", bufs=1) as acc_pool,
            tc.tile_pool(name="scr", bufs=SCR_BUFS) as scr_pool,
            tc.tile_pool(name="psum_z", bufs=Z_BUFS, space="PSUM") as z_pool,
            tc.tile_pool(name="psum_d", bufs=DOT_BUFS, space="PSUM") as d_pool,
        ):
            dgrp = 2 if DOTS_QUAD else 1
            dbase = DOTS_BASE
            if DOTS_DR:
                # d = r*32 + p packing on both operands (any consistent
                # bijection works; the PE pairs stationary row (p, r) with
                # moving (p, r)).
                fjT_sb = feat_pool.tile([dbase + KD // 2, Q, 2, N], fp8)
                fiT_sb = feat_pool.tile(
                    [dbase + KD // 2, Q, NT, 2, wmax], fp8
                )
            else:
                fjT_sb = feat_pool.tile(
                    [dbase + KD if dbase else KD, Q, N], fp8
                )
                fiT_sb = feat_pool.tile(
                    [dbase + KD if dbase else KD, Q, NT, wmax], fp8
                )
            ngrp = D2_NGRP if D2_QUAD else 1
            geoL_sb = geo_pool.tile([P if D2_QUAD else KGEO, Q, N], bf16)
            geoR_sb = geo_pool.tile(
                [P if D2_QUAD else KGEO, Q, NT, wmax], bf16
            )
            cnt_acc = acc_pool.tile([P, nseg], f32)
            ext_acc = acc_pool.tile([P, nseg], f32)
            if mode != "full":
                nc.vector.memset(cnt_acc[:], 0.0)
                nc.vector.memset(ext_acc[:], 0.0)

            # Input DMAs, split per slot so slot-0's first matmuls only wait
            # on slot-0 data (halves the startup dependency).
            for qq in range(Q):
                for rg in range(ngrp):
                    nc.sync.dma_start(
                        out=geoL_sb[32 * rg : 32 * rg + KGEO, qq : qq + 1, :],
                        in_=geoL_d[qq : qq + 1].rearrange("q k n -> k q n"),
                    )
                    nc.sync.dma_start(
                        out=geoR_sb[
                            32 * rg : 32 * rg + KGEO, qq : qq + 1, :, :
                        ],
                        in_=geoR_d[qq : qq + 1].rearrange(
                            "q t k w -> k q t w"
                        ),
                    )
            if DOTS_DR:
                nc.sync.dma_start(
                    out=fjT_sb[dbase : dbase + KD // 2, :, :, :],
                    in_=fjT_d[:].rearrange("q (r p) n -> p q r n", p=KD // 2),
                )
                nc.sync.dma_start(
                    out=fiT_sb[dbase : dbase + KD // 2, :, :, :, :],
                    in_=fiT_d[:].rearrange(
                        "q t (r p) w -> p q t r w", p=KD // 2
                    ),
                )
            else:
                for qq in range(Q):
                    for dg in range(dgrp):
                        o = dbase + KD * dg
                        nc.sync.dma_start(
                            out=fjT_sb[o : o + KD, qq : qq + 1, :],
                            in_=fjT_d[qq : qq + 1].rearrange("q k n -> k q n"),
                        )
                        nc.sync.dma_start(
                            out=fiT_sb[o : o + KD, qq : qq + 1, :, :],
                            in_=fiT_d[qq : qq + 1].rearrange(
                                "q t k w -> k q t w"
                            ),
                        )

            # No DMA-tick absorbers: the first z matmul should start as soon
            # as the (small) geometry DMAs land, without waiting for the
            # larger fjT/fiT transfers; the first dots matmul carries those
            # waits naturally (hoisted by _split_multi_waits where needed).
            if mode == "noop":
                dummy_sb = scr_pool.tile([1, 8], f32, tag="dmy")
                nc.vector.memset(dummy_sb[:], 0.0)

            def emit_seg(si, q, blocks, W):
                z_ps = z_pool.tile([P, W], f32, tag="z")
                d_ps = d_pool.tile([P, W], f32, tag="d")
                if mode in ("full", "d2only", "zvec"):
                    # One PE row group per segment: matmuls within a segment
                    # share a PSUM bank (must be serial); consecutive
                    # segments use different row groups and banks, so their
                    # d2 streams overlap on the PE array.
                    rg = (si % ngrp) * 32
                    for nt, wlo, wlen, coff in blocks:
                        nc.tensor.matmul(
                            z_ps[:, coff : coff + wlen],
                            geoL_sb[rg : rg + KGEO, q, nt * P : (nt + 1) * P],
                            geoR_sb[rg : rg + KGEO, q, nt, wlo : wlo + wlen],
                            start=True,
                            stop=True,
                            tile_position=(rg, 0),
                        )
                if mode in ("full", "dotsonly"):
                    if DOTS_DR:
                        for nt, wlo, wlen, coff in blocks:
                            nc.tensor.matmul(
                                d_ps[:, coff : coff + wlen],
                                fjT_sb[dbase : dbase + KD // 2, q, 0:2,
                                       nt * P : (nt + 1) * P],
                                fiT_sb[dbase : dbase + KD // 2, q, nt, 0:2,
                                       wlo : wlo + wlen],
                                start=True,
                                stop=True,
                                perf_mode=mybir.MatmulPerfMode.DoubleRow,
                            )
                    else:
                        dg = dbase + (si % dgrp) * KD
                        for nt, wlo, wlen, coff in blocks:
                            nc.tensor.matmul(
                                d_ps[:, coff : coff + wlen],
                                fjT_sb[dg : dg + KD, q,
                                       nt * P : (nt + 1) * P],
                                fiT_sb[dg : dg + KD, q, nt,
                                       wlo : wlo + wlen],
                                start=True,
                                stop=True,
                            )
                if mode in ("full", "zvec"):
                    sgn_scr = scr_pool.tile([P, W], bf16, tag="sgn")
                    nc.scalar.activation(
                        sgn_scr[:],
                        z_ps[:],
                        mybir.ActivationFunctionType.Sign,
                        accum_out=cnt_acc[:, si : si + 1],
                    )
                if mode == "full":
                    stt_scr = scr_pool.tile([P, W], bf16, tag="stt")
                    if STT_DUAL_PSUM:
                        nc.vector.scalar_tensor_tensor(
                            out=stt_scr[:],
                            in0=z_ps[:],
                            scalar=0.0,
                            in1=d_ps[:],
                            op0=mybir.AluOpType.is_ge,
                            op1=mybir.AluOpType.mult,
                            accum_out=ext_acc[:, si : si + 1],
                        )
                    else:
                        nc.vector.scalar_tensor_tensor(
                            out=stt_scr[:],
                            in0=sgn_scr[:],
                            scalar=0.0,
                            in1=d_ps[:],
                            op0=mybir.AluOpType.bypass,
                            op1=mybir.AluOpType.mult,
                            accum_out=ext_acc[:, si : si + 1],
                        )

            def emit_body():
                if mode == "noop":
                    nc.scalar.copy(dummy_sb[0:1, 2:3], dummy_sb[0:1, 0:1])
                    return
                for si, (q, blocks, W) in enumerate(segs):
                    emit_seg(si, q, blocks, W)

            if reps == 1:
                emit_body()
            else:
                with tc.For_i(0, reps, 1):
                    emit_body()

            nc.sync.dma_start(out=out_d[:, 0:nseg], in_=cnt_acc[:])
            nc.sync.dma_start(out=out_d[:, nseg : 2 * nseg], in_=ext_acc[:])

    if split_waits:
        _split_multi_waits(nc)
    return nc


def _split_multi_waits(nc):
    """Walrus rejects >1 sync-wait on compute/DMA instruction encodings.

    Hoist all but one wait of any multi-wait instruction onto standalone
    InstEventSemaphore instructions inserted immediately before it on the
    same engine queue.
    """
    import concourse.mybir as mybir

    n_split = 0
    for bb in nc.main_func.blocks:
        new_list = []
        for inst in bb.instructions:
            si = inst.sync_info
            if (
                si is not None
                and si.on_wait
                and len(si.on_wait) > 1
                and not isinstance(inst, mybir.InstEventSemaphore)
            ):
                waits = list(si.on_wait)
                for k, w in enumerate(waits[:-1]):
                    n_split += 1
                    new_list.append(
                        mybir.InstEventSemaphore(
                            name=f"{inst.name}-hw{k}",
                            engine=inst.engine,
                            ins=[],
                            outs=[],
                            sync_info=mybir.SyncInfo(on_wait=[w], on_update=[]),
                        )
                    )
                inst.sync_info = mybir.SyncInfo(
                    on_wait=[waits[-1]], on_update=list(si.on_update or [])
                )
            new_list.append(inst)
        bb.instructions[:] = new_list
    return n_split


def _get_bass(sched):
    key = ("nc", sched["segs"], STT_DUAL_PSUM)
    if key not in _CACHE:
        _CACHE[key] = _build_bass(sched)
    return _CACHE[key]


def _combine(results, sched, in_maps=None):
    segs = sched["segs"]
    nseg = sched["nseg"]
    a_tot = 0.0
    b_tot = 0.0
    for c, res in enumerate(results):
        out = res["out"].astype(np.float64)
        cnt = out[:, 0:nseg].sum(axis=0)
        ext = out[:, nseg : 2 * nseg].sum(axis=0)
        for si, (q, blocks, W) in enumerate(segs):
            a_tot += 0.5 * (cnt[si] + P * W)
            if STT_DUAL_PSUM:
                b_tot += ext[si]
            else:
                # +-1 ext convention: b = 0.5*(ext + sum_window dots)
                corr = 0.0
                fj = in_maps[c]["fjT"][q].astype(np.float64)   # [KD, N]
                fi = in_maps[c]["fiT"][q].astype(np.float64)   # [NT,KD,wmax]
                for nt, wlo, wlen, coff in blocks:
                    cj = fj[:, nt * P : (nt + 1) * P].sum(axis=1)
                    ci = fi[nt, :, wlo : wlo + wlen].sum(axis=1)
                    corr += float(cj @ ci)
                b_tot += 0.5 * (ext[si] + corr)
    return a_tot, b_tot


def kernel(features, pts_src, pts_dst, invis_idx, height, width):
    global LAST
    del invis_idx  # unused by the reference computation

    features = np.asarray(features)
    pts_src = np.asarray(pts_src)
    pts_dst = np.asarray(pts_dst)

    in_maps, sched = _host_prep(features, pts_src, pts_dst, height, width)

    from concourse.bass_utils import run_bass_kernel_spmd

    nc = _get_bass(sched)
    LAST = run_bass_kernel_spmd(nc, in_maps, core_ids=list(range(N_CORES)))

    a_tot, b_tot = _combine(LAST.results, sched, in_maps)
    loss = (a_tot - b_tot) / max(a_tot, 1.0)
    return np.float32(loss)
